# revision 17
# baseline (speedup 1.0000x reference)
"""Trainium2 Bass kernel for nn_MultiHeadAttention (B=2, S=2048, D=1024, H=16).

Sharding: 8 cores = 2 (batch) x 4 (head groups of 4 heads / 256 dims).
Each core computes QKV projections for its head slice, attention for its 4
heads, and the partial output projection for its 256-dim slice of Wo's input.
Host sums the 4 partials per batch element (Megatron-style row-parallel Wo).

Device layouts (per core):
  qT/kT/vT  [1024, 2048] bf16   (input, transposed on host)
  wqT/wkT/wvT [1024, 256] bf16  (Wq[js].T etc)
  woT       [256, 1024] bf16    (Wo[:, js].T)
  maskT     [2048, 2048] bf16   (mask[0,0].T as 0.0/1.0)
  qpT/kpT   [256(j), 2048(s)]   (projections, transposed: j on partitions)
  vp        [2048(t), 4x65]     (natural layout; col 64 of each 65-block = 1.0
                                 -> attn@V matmul also produces softmax denom)
  P~        [t, s] = exp(scoresT/8) * maskT   (scoresT = K_h.T^T @ Q_h.T)
  attn out  [65(j+denom), s] -> normalized -> concatT [256(j), 2048(s)]
  out_p     [2048, 1024] f32 partial = concatT.T @ woT
"""

import sys

import numpy as np

try:
    import concourse.bass as bass
except ImportError:  # pragma: no cover
    sys.path.insert(0, "/opt/trn_rl_repo")
    import concourse.bass as bass

from concourse import bacc

import ml_dtypes

import concourse.tile as tile_mod
from concourse import mybir
from concourse.bass_utils import run_bass_kernel_spmd

BF16 = ml_dtypes.bfloat16
F32 = np.float32

B, S, D, H = 2, 2048, 1024, 16
DK = D // H            # 64
N_CORES = 8
HPC = 4                # heads per core
JC = HPC * DK          # 256 j-dims per core
SCALE = 1.0 / float(np.sqrt(DK))
NSB = S // 512         # 4 s-blocks
NC_T = S // 128        # 16 t-chunks
VROW = HPC * 65        # 260: [h0 64 | 1 | h1 64 | 1 | ...]

bf = mybir.dt.bfloat16
f32 = mybir.dt.float32


def _patch_drain():
    """This walrus build only accepts 1 sync-wait per instruction; the Tile
    exit drain carries one wait per pending proc. Split them across drains."""
    if getattr(tile_mod.TileContext, "_drain_patched", False):
        return
    import bass_rust

    def _drain_and_barrier(self, tick_clock, wait_clock):
        from concourse.tile import ScopedClock

        nc = self.nc
        drain_inst = nc.sync.drain()
        wait_clock.add_sem_waits(
            drain_inst.ins, ScopedClock({None: tick_clock.global_clock})
        )
        si = drain_inst.ins.sync_info
        waits = list(si.on_wait)
        if len(waits) > 1:
            drain_inst.ins.sync_info = bass_rust.SyncInfo(
                on_wait=[waits[0]], on_update=list(si.on_update)
            )
            for w in waits[1:]:
                d2 = nc.sync.drain()
                d2.ins.sync_info = bass_rust.SyncInfo(on_wait=[w], on_update=[])
        nc.all_engine_barrier()
        assert self.sems is not None
        popped = nc._tile_sem_poison_stack.pop()
        assert popped is self._sem_poison
        nc.clear_and_free_semaphores(list(self.sems.allocated().values()))
        nc.all_engine_barrier()

    tile_mod.TileContext._drain_and_barrier = _drain_and_barrier
    tile_mod.TileContext._drain_patched = True


def _emit(tc, T):
    nc = tc.nc
    Exp = mybir.ActivationFunctionType.Exp

    from contextlib import ExitStack

    with ExitStack() as ctx:
        persist = ctx.enter_context(tc.tile_pool(name="persist", bufs=1))

        # ---- weights / persistent tiles ----
        wq = persist.tile([128, 8 * JC], bf, tag="wq")
        wk = persist.tile([128, 8 * JC], bf, tag="wk")
        wv = persist.tile([128, 8 * JC], bf, tag="wv")
        for t, name in ((wq, "wqT"), (wk, "wkT"), (wv, "wvT")):
            nc.sync.dma_start(
                t[:].rearrange("p (c j) -> p c j", c=8),
                T[name][:, :].rearrange("(c p) j -> p c j", p=128),
            )
        wo = [persist.tile([128, D], bf, tag=f"wo{i}", name=f"wo{i}") for i in range(2)]
        for i in range(2):
            nc.sync.dma_start(wo[i][:], T["woT"][i * 128 : (i + 1) * 128, :])
        biasqk = persist.tile([128, 4], f32, tag="biasqk")
        nc.sync.dma_start(biasqk[:], T["biasqk"][:, :])

        # per-sb q/k projection tiles ([j, s] transposed layout)
        qpS = [
            [persist.tile([128, 512], bf, tag=f"qp{j}_{s}", name=f"qp{j}_{s}")
             for s in range(NSB)]
            for j in range(2)
        ]
        kpT = [
            [persist.tile([128, 1024], bf, tag=f"kpT{i}_{th}", name=f"kpT{i}_{th}")
             for th in range(2)]
            for i in range(2)
        ]
        # per-chunk v tiles (natural [t, j] layout + ones cols)
        vpc = [persist.tile([128, VROW], bf, tag=f"vp{c}", name=f"vp{c}")
               for c in range(NC_T)]
        concatT = [persist.tile([128, S], bf, tag=f"concatT{i}", name=f"concatT{i}") for i in range(2)]

        wq_v = wq[:].rearrange("p (c j) -> p c j", c=8)
        wk_v = wk[:].rearrange("p (c j) -> p c j", c=8)
        wv_v = wv[:].rearrange("p (c j) -> p c j", c=8)

        q_stream = ctx.enter_context(tc.tile_pool(name="q_stream", bufs=2))
        qtts = {}

        def emit_qdma(sb):
            sl = slice(sb * 512, (sb + 1) * 512)
            qTt = q_stream.tile([128, 8 * 512], bf, tag="qTt", name=f"qTt{sb}")
            nc.sync.dma_start(
                qTt[:].rearrange("p (c s) -> p c s", c=8),
                T["qT"][:, sl].rearrange("(c p) s -> p c s", p=128),
            )
            qtts[sb] = qTt[:].rearrange("p (c s) -> p c s", c=8)

        def emit_qproj_jt(sb, jt):
            jsl = slice(jt * 128, (jt + 1) * 128)
            ps = bigp.tile([128, 512], f32, tag="big", name=f"pq{sb}_{jt}")
            for c in range(8):
                nc.tensor.matmul(
                    ps[:], wq_v[:, c, jsl], qtts[sb][:, c, :],
                    start=(c == 0), stop=(c == 7),
                )
            nc.vector.tensor_scalar_add(
                qpS[jt][sb][:], ps[:], biasqk[:, jt : jt + 1]
            )

        def emit_qproj(sb):
            emit_qdma(sb)
            emit_qproj_jt(sb, 0)
            emit_qproj_jt(sb, 1)

        # ---- q(0) + k projections (scores need all of kpT) ----
        bigp = ctx.enter_context(tc.tile_pool(name="bigp", bufs=4, space="PSUM"))
        emit_qproj(0)
        with tc.tile_pool(name="kv_stream", bufs=4) as kv_stream:
            kttv = []
            for sb in range(NSB):
                sl = slice(sb * 512, (sb + 1) * 512)
                kTt = kv_stream.tile([128, 8 * 512], bf, tag="kTt", name=f"kTt{sb}")
                nc.sync.dma_start(
                    kTt[:].rearrange("p (c s) -> p c s", c=8),
                    T["kT"][:, sl].rearrange("(c p) s -> p c s", p=128),
                )
                kttv.append(kTt[:].rearrange("p (c s) -> p c s", c=8))
            for sb in range(NSB):
                kTt_v = kttv[sb]
                for jt in range(2):
                    jsl = slice(jt * 128, (jt + 1) * 128)
                    ps = bigp.tile([128, 512], f32, tag="big", name=f"pk{sb}_{jt}")
                    for c in range(8):
                        nc.tensor.matmul(
                            ps[:], wk_v[:, c, jsl], kTt_v[:, c, :],
                            start=(c == 0), stop=(c == 7),
                        )
                    nc.vector.tensor_scalar_add(
                        kpT[jt][sb // 2][:, (sb % 2) * 512 : (sb % 2 + 1) * 512],
                        ps[:], biasqk[:, 2 + jt : 3 + jt]
                    )

        # ---- attention + output projection ----
        # Chunk-level software pipeline: per t-chunk the PE stream carries
        # scores(i) for both heads (concurrent row-groups), then attnV(i-1)
        # for both heads, plus occasional "extras" (Wo / q-proj / v-proj /
        # mask prefetch). ACT (exp) is the pacing engine; this keeps it fed
        # every chunk while the PE stays dense enough to hold HAM at 8/8.
        with (
            tc.tile_pool(name="vstream", bufs=3) as vstream,
            tc.tile_pool(name="maskp", bufs=2) as maskp,
            tc.tile_pool(name="ptp", bufs=2) as ptp,
            tc.tile_pool(name="smallp", bufs=2) as smallp,
            tc.tile_pool(name="outp", bufs=3) as outp,
            tc.tile_pool(name="scp", bufs=2, space="PSUM") as scp,
        ):
            mtiles = {}

            def emit_mask_dma(sb):
                sl = slice(sb * 512, (sb + 1) * 512)
                mT = maskp.tile([128, NC_T * 512], bf, tag="mT", name=f"mT{sb}")
                nc.sync.dma_start(
                    mT[:].rearrange("p (c s) -> p c s", c=NC_T),
                    T["maskT"][:, sl].rearrange("(c p) s -> p c s", p=128),
                )
                mtiles[sb] = mT

            vtts = {}

            def emit_vdma(tb):
                sl = slice(tb * 512, (tb + 1) * 512)
                vTt = vstream.tile([128, 8 * 512], bf, tag="vTt", name=f"vTt{tb}")
                nc.sync.dma_start(
                    vTt[:].rearrange("p (c s) -> p c s", c=8),
                    T["vT"][:, sl].rearrange("(c p) s -> p c s", p=128),
                )
                vtts[tb] = vTt[:].rearrange("p (c t) -> p c t", c=8)

            def emit_vproj_tb(tb):
                vTt_v = vtts[tb]
                if tb + 2 < NSB:
                    emit_vdma(tb + 2)
                for tt in range(4):
                    chunk = tb * 4 + tt
                    ps = bigp.tile([128, 512], f32, tag="big", name=f"pv{chunk}")
                    for c in range(8):
                        nc.tensor.matmul(
                            ps[:, 0:JC],
                            vTt_v[:, c, tt * 128 : (tt + 1) * 128],
                            wv_v[:, c, :],
                            start=(c == 0), stop=(c == 7),
                        )
                    vt = vpc[chunk]
                    nc.gpsimd.memset(
                        vt[:].rearrange("p (h d) -> p h d", d=65)[:, :, 64:65],
                        1.0,
                    )
                    dst = vt[:].rearrange("p (h d) -> p h d", h=HPC)[:, :, 0:DK]
                    src = ps[:, 0:JC].rearrange("p (h d) -> p h d", h=HPC)
                    nc.vector.tensor_copy(dst, src)

            def emit_wo_group(sb, st, mt):
                s0 = sb * 512 + st * 128
                msl = slice(mt * 512, (mt + 1) * 512)
                pw = bigp.tile([128, 512], f32, tag="big", name=f"pw{sb}_{st}_{mt}")
                for kc in range(2):
                    nc.tensor.matmul(
                        pw[:],
                        concatT[kc][:, s0 : s0 + 128],
                        wo[kc][:, msl],
                        start=(kc == 0), stop=(kc == 1),
                    )
                ot = outp.tile([128, 512], f32, tag="ot", name=f"ot{sb}_{st}_{mt}")
                nc.vector.tensor_copy(ot[:], pw[:])
                nc.sync.dma_start(T["out_p"][s0 : s0 + 128, msl], ot[:])

            def emit_norm(sb, pair, po2):
                sl = slice(sb * 512, (sb + 1) * 512)
                for h2 in range(2):
                    h = pair * 2 + h2
                    psl = slice(h2 * 64, h2 * 64 + 64)
                    po = po2[h2]
                    rc0 = smallp.tile([1, 512], f32, tag="rc0", name=f"rc0_{sb}_{h}")
                    nc.vector.tensor_copy(rc0[:], po[64:65, :])
                    rc = smallp.tile([1, 512], f32, tag="rc", name=f"rc{sb}_{h}")
                    nc.vector.reciprocal_approx_fast(rc[:], rc0[:])
                    rb = smallp.tile([64, 512], f32, tag="rb", name=f"rb{sb}_{h}")
                    nc.gpsimd.partition_broadcast(rb[:], rc[:], channels=64)
                    nc.vector.tensor_mul(
                        concatT[pair][psl, sl], po[0:64, :], rb[:]
                    )

            emit_mask_dma(0)
            emit_vdma(0)
            emit_vdma(1)
            extras = []
            po2L = None
            prev = None        # (sb, pair, Pt, po2)
            for sb in range(NSB):
                for pair in range(2):
                    last_it = (sb == NSB - 1 and pair == 1)
                    if sb == 0:
                        extras.append(lambda t=2 * pair: emit_vproj_tb(t))
                        extras.append(lambda t=2 * pair + 1: emit_vproj_tb(t))

                    Pt = ptp.tile(
                        [128, 2 * NC_T * 512], bf, tag="Pt", name=f"Pt{sb}_{pair}"
                    )
                    pv = Pt[:].rearrange("p (c h s) -> p c h s", c=NC_T, h=2)
                    mv = mtiles[sb][:].rearrange("p (c s) -> p c s", c=NC_T)
                    if prev is not None:
                        po2 = [
                            bigp.tile([128, 512], f32, tag="big",
                                      name=f"av{prev[0]}_{prev[1]}_{h2}")
                            for h2 in range(2)
                        ]
                    for c in range(NC_T):
                        ps = scp.tile(
                            [128, 1024], f32, tag="sc", name=f"sc{sb}_{pair}_{c}"
                        )
                        for h2 in range(2):
                            psl = slice(h2 * 64, h2 * 64 + 64)
                            nc.tensor.matmul(
                                ps[:, h2 * 512 : (h2 + 1) * 512],
                                kpT[pair][c // 8][psl, (c % 8) * 128 : (c % 8 + 1) * 128],
                                qpS[pair][sb][psl, :],
                                start=True, stop=True,
                            )
                        nc.scalar.activation(
                            Pt[:, c * 1024 : (c + 1) * 1024],
                            ps[:], Exp, scale=SCALE,
                        )
                        if prev is not None:
                            psb, ppair, pPt, _ = prev
                            for h2 in range(2):
                                h = ppair * 2 + h2
                                nc.tensor.matmul(
                                    po2[h2][0:65, :],
                                    vpc[c][:, h * 65 : h * 65 + 65],
                                    pPt[:, (2 * c + h2) * 512 : (2 * c + h2 + 1) * 512],
                                    start=(c == 0), stop=(c == NC_T - 1),
                                )
                        if last_it and c >= 8:
                            if c == 8:
                                po2L = [
                                    bigp.tile([128, 512], f32, tag="big",
                                              name=f"avL_{h2}")
                                    for h2 in range(2)
                                ]
                            cc = c - 8
                            for h2 in range(2):
                                h = pair * 2 + h2
                                nc.tensor.matmul(
                                    po2L[h2][0:65, :],
                                    vpc[cc][:, h * 65 : h * 65 + 65],
                                    Pt[:, (2 * cc + h2) * 512 : (2 * cc + h2 + 1) * 512],
                                    start=(cc == 0), stop=False,
                                )
                        if c == 7 or c == NC_T - 1:
                            half = slice(0, 8) if c == 7 else slice(8, NC_T)
                            for h2 in range(2):
                                nc.vector.tensor_mul(
                                    pv[:, half, h2, :], pv[:, half, h2, :],
                                    mv[:, half, :],
                                )
                        if c == 1 and pair == 0 and sb + 1 < NSB:
                            emit_mask_dma(sb + 1)
                            emit_qdma(sb + 1)
                        elif c in (1, 3) and pair == 1 and sb + 1 < NSB:
                            emit_qproj_jt(sb + 1, c // 2)
                        elif extras and c % 2 == 1 and c >= 5:
                            extras.pop(0)()
                    if prev is not None:
                        emit_norm(prev[0], prev[1], po2)
                        if prev[1] == 1:
                            for st in range(4):
                                for mt in range(2):
                                    extras.append(
                                        lambda s=prev[0], a=st, b=mt:
                                        emit_wo_group(s, a, b)
                                    )
                    prev = (sb, pair, Pt, None)
            # tail: finish attnv(3,1) chunks 8..15, then norm + final Wo
            psb, ppair, pPt, _ = prev
            for c in range(8, NC_T):
                for h2 in range(2):
                    h = ppair * 2 + h2
                    nc.tensor.matmul(
                        po2L[h2][0:65, :],
                        vpc[c][:, h * 65 : h * 65 + 65],
                        pPt[:, (2 * c + h2) * 512 : (2 * c + h2 + 1) * 512],
                        start=False, stop=(c == NC_T - 1),
                    )
                if extras and c % 2 == 1:
                    extras.pop(0)()
            emit_norm(psb, ppair, po2L)
            for fn in extras:
                fn()
            for st in range(4):
                for mt in range(2):
                    emit_wo_group(NSB - 1, st, mt)


def build_nc():
    nc = bacc.Bacc("TRN2", target_bir_lowering=False, debug=False)
    names = {}
    def din(name, shape, dt):
        names[name] = nc.dram_tensor(name, shape, dt, kind="ExternalInput").ap()
    din("qT", [D, S], bf)
    din("kT", [D, S], bf)
    din("vT", [D, S], bf)
    din("maskT", [S, S], bf)
    din("wqT", [D, JC], bf)
    din("wkT", [D, JC], bf)
    din("wvT", [D, JC], bf)
    din("woT", [JC, D], bf)
    din("biasqk", [128, 4], f32)
    names["out_p"] = nc.dram_tensor(
        "out_p", [S, D], f32, kind="ExternalOutput"
    ).ap()
    with tile_mod.TileContext(nc) as tc:
        _emit(tc, names)
    nc.compile()
    return nc


_NC = None


def prep_inputs(q, k, v, mask, Wq, bq, Wk, bk, Wv, bv, Wo, bo):
    q = np.asarray(q, F32)
    k = np.asarray(k, F32)
    v = np.asarray(v, F32)
    mask = np.asarray(mask)
    Wq, Wk, Wv, Wo = (np.asarray(w, F32) for w in (Wq, Wk, Wv, Wo))
    bq, bk, bv, bo = (np.asarray(b_, F32) for b_ in (bq, bk, bv, bo))

    maskT = np.ascontiguousarray(mask[0, 0].T).astype(BF16)
    qT = [np.ascontiguousarray(q[b_].T).astype(BF16) for b_ in range(B)]
    kT = [np.ascontiguousarray(k[b_].T).astype(BF16) for b_ in range(B)]
    vT = [np.ascontiguousarray(v[b_].T).astype(BF16) for b_ in range(B)]

    in_maps = []
    for c in range(N_CORES):
        b_, g = c // 4, c % 4
        js = slice(g * JC, (g + 1) * JC)
        biasqk = np.stack(
            [bq[js][:128], bq[js][128:], bk[js][:128], bk[js][128:]], axis=1
        ).astype(F32)
        in_maps.append(
            {
                "qT": qT[b_],
                "kT": kT[b_],
                "vT": vT[b_],
                "maskT": maskT,
                "wqT": np.ascontiguousarray(Wq[js, :].T).astype(BF16),
                "wkT": np.ascontiguousarray(Wk[js, :].T).astype(BF16),
                "wvT": np.ascontiguousarray(Wv[js, :].T).astype(BF16),
                "woT": np.ascontiguousarray(Wo[:, js].T).astype(BF16),
                "biasqk": np.ascontiguousarray(biasqk),
            }
        )
    # bv contributes a constant (softmax rows sum to 1): out += Wo @ bv + bo
    bias_out = (Wo @ bv + bo).astype(F32)
    return in_maps, bias_out


def run_prepped(in_maps, bias_out, trace=False, **kw):
    global _NC
    if _NC is None:
        _NC = build_nc()
    res = run_bass_kernel_spmd(
        _NC, in_maps, list(range(N_CORES)), trace=trace, **kw
    )
    out = np.zeros((B, S, D), F32)
    for c in range(N_CORES):
        out[c // 4] += res.results[c]["out_p"]
    out += bias_out[None, None, :]
    return out, res


def kernel(q, k, v, mask, Wq, bq, Wk, bk, Wv, bv, Wo, bo):
    in_maps, bias_out = prep_inputs(
        q, k, v, mask, Wq, bq, Wk, bk, Wv, bv, Wo, bo
    )
    out, _ = run_prepped(in_maps, bias_out)
    return out


# revision 19
# speedup vs baseline: 1.0130x; 1.0130x over previous
"""Trainium2 Bass kernel for nn_MultiHeadAttention (B=2, S=2048, D=1024, H=16).

Sharding: 8 cores = 2 (batch) x 4 (head groups of 4 heads / 256 dims).
Each core computes QKV projections for its head slice, attention for its 4
heads, and the partial output projection for its 256-dim slice of Wo's input.
Host sums the 4 partials per batch element (Megatron-style row-parallel Wo).

Device layouts (per core):
  qT/kT/vT  [1024, 2048] bf16   (input, transposed on host)
  wqT/wkT/wvT [1024, 256] bf16  (Wq[js].T etc)
  woT       [256, 1024] bf16    (Wo[:, js].T)
  maskT     [2048, 2048] bf16   (mask[0,0].T as 0.0/1.0)
  qpT/kpT   [256(j), 2048(s)]   (projections, transposed: j on partitions)
  vp        [2048(t), 4x65]     (natural layout; col 64 of each 65-block = 1.0
                                 -> attn@V matmul also produces softmax denom)
  P~        [t, s] = exp(scoresT/8) * maskT   (scoresT = K_h.T^T @ Q_h.T)
  attn out  [65(j+denom), s] -> normalized -> concatT [256(j), 2048(s)]
  out_p     [2048, 1024] f32 partial = concatT.T @ woT
"""

import sys

import numpy as np

try:
    import concourse.bass as bass
except ImportError:  # pragma: no cover
    sys.path.insert(0, "/opt/trn_rl_repo")
    import concourse.bass as bass

from concourse import bacc

import ml_dtypes

import concourse.tile as tile_mod
from concourse import mybir
from concourse.bass_utils import run_bass_kernel_spmd

BF16 = ml_dtypes.bfloat16
F32 = np.float32

B, S, D, H = 2, 2048, 1024, 16
DK = D // H            # 64
N_CORES = 8
HPC = 4                # heads per core
JC = HPC * DK          # 256 j-dims per core
SCALE = 1.0 / float(np.sqrt(DK))
NSB = S // 512         # 4 s-blocks
NC_T = S // 128        # 16 t-chunks
VROW = HPC * 65        # 260: [h0 64 | 1 | h1 64 | 1 | ...]

bf = mybir.dt.bfloat16
f32 = mybir.dt.float32


def _patch_drain():
    """This walrus build only accepts 1 sync-wait per instruction; the Tile
    exit drain carries one wait per pending proc. Split them across drains."""
    if getattr(tile_mod.TileContext, "_drain_patched", False):
        return
    import bass_rust

    def _drain_and_barrier(self, tick_clock, wait_clock):
        from concourse.tile import ScopedClock

        nc = self.nc
        drain_inst = nc.sync.drain()
        wait_clock.add_sem_waits(
            drain_inst.ins, ScopedClock({None: tick_clock.global_clock})
        )
        si = drain_inst.ins.sync_info
        waits = list(si.on_wait)
        if len(waits) > 1:
            drain_inst.ins.sync_info = bass_rust.SyncInfo(
                on_wait=[waits[0]], on_update=list(si.on_update)
            )
            for w in waits[1:]:
                d2 = nc.sync.drain()
                d2.ins.sync_info = bass_rust.SyncInfo(on_wait=[w], on_update=[])
        nc.all_engine_barrier()
        assert self.sems is not None
        popped = nc._tile_sem_poison_stack.pop()
        assert popped is self._sem_poison
        nc.clear_and_free_semaphores(list(self.sems.allocated().values()))
        nc.all_engine_barrier()

    tile_mod.TileContext._drain_and_barrier = _drain_and_barrier
    tile_mod.TileContext._drain_patched = True


def _emit(tc, T):
    nc = tc.nc
    Exp = mybir.ActivationFunctionType.Exp

    from contextlib import ExitStack

    with ExitStack() as ctx:
        persist = ctx.enter_context(tc.tile_pool(name="persist", bufs=1))

        # ---- weights / persistent tiles ----
        wq = persist.tile([128, 8 * JC], bf, tag="wq")
        wk = persist.tile([128, 8 * JC], bf, tag="wk")
        wv = persist.tile([128, 8 * JC], bf, tag="wv")
        for t, name in ((wq, "wqT"), (wk, "wkT"), (wv, "wvT")):
            nc.scalar.dma_start(
                t[:].rearrange("p (c j) -> p c j", c=8),
                T[name][:, :].rearrange("(c p) j -> p c j", p=128),
            )
        wo = [persist.tile([128, D], bf, tag=f"wo{i}", name=f"wo{i}") for i in range(2)]
        for i in range(2):
            nc.scalar.dma_start(wo[i][:], T["woT"][i * 128 : (i + 1) * 128, :])
        biasqk = persist.tile([128, 4], f32, tag="biasqk")
        nc.scalar.dma_start(biasqk[:], T["biasqk"][:, :])

        # per-sb q/k projection tiles ([j, s] transposed layout)
        qpS = [
            [persist.tile([128, 512], bf, tag=f"qp{j}_{s}", name=f"qp{j}_{s}")
             for s in range(NSB)]
            for j in range(2)
        ]
        kpT = [
            [persist.tile([128, 1024], bf, tag=f"kpT{i}_{th}", name=f"kpT{i}_{th}")
             for th in range(2)]
            for i in range(2)
        ]
        # per-chunk v tiles (natural [t, j] layout + ones cols)
        vpc = [persist.tile([128, VROW], bf, tag=f"vp{c}", name=f"vp{c}")
               for c in range(NC_T)]
        concatT = [persist.tile([128, S], bf, tag=f"concatT{i}", name=f"concatT{i}") for i in range(2)]

        wq_v = wq[:].rearrange("p (c j) -> p c j", c=8)
        wk_v = wk[:].rearrange("p (c j) -> p c j", c=8)
        wv_v = wv[:].rearrange("p (c j) -> p c j", c=8)

        q_stream = ctx.enter_context(tc.tile_pool(name="q_stream", bufs=2))
        qtts = {}

        def emit_qdma(sb):
            sl = slice(sb * 512, (sb + 1) * 512)
            qTt = q_stream.tile([128, 8 * 512], bf, tag="qTt", name=f"qTt{sb}")
            nc.sync.dma_start(
                qTt[:].rearrange("p (c s) -> p c s", c=8),
                T["qT"][:, sl].rearrange("(c p) s -> p c s", p=128),
            )
            qtts[sb] = qTt[:].rearrange("p (c s) -> p c s", c=8)

        def emit_qproj_jt(sb, jt):
            jsl = slice(jt * 128, (jt + 1) * 128)
            ps = bigp.tile([128, 512], f32, tag="big", name=f"pq{sb}_{jt}")
            for c in range(8):
                nc.tensor.matmul(
                    ps[:], wq_v[:, c, jsl], qtts[sb][:, c, :],
                    start=(c == 0), stop=(c == 7),
                )
            nc.vector.tensor_scalar_add(
                qpS[jt][sb][:], ps[:], biasqk[:, jt : jt + 1]
            )

        def emit_qproj(sb):
            emit_qdma(sb)
            emit_qproj_jt(sb, 0)
            emit_qproj_jt(sb, 1)

        # ---- q(0) + k projections (scores need all of kpT) ----
        bigp = ctx.enter_context(tc.tile_pool(name="bigp", bufs=4, space="PSUM"))
        emit_qproj(0)
        with tc.tile_pool(name="kv_stream", bufs=4) as kv_stream:
            kttv = []
            for sb in range(NSB):
                sl = slice(sb * 512, (sb + 1) * 512)
                kTt = kv_stream.tile([128, 8 * 512], bf, tag="kTt", name=f"kTt{sb}")
                nc.sync.dma_start(
                    kTt[:].rearrange("p (c s) -> p c s", c=8),
                    T["kT"][:, sl].rearrange("(c p) s -> p c s", p=128),
                )
                kttv.append(kTt[:].rearrange("p (c s) -> p c s", c=8))
            for sb in range(NSB):
                kTt_v = kttv[sb]
                for jt in range(2):
                    jsl = slice(jt * 128, (jt + 1) * 128)
                    ps = bigp.tile([128, 512], f32, tag="big", name=f"pk{sb}_{jt}")
                    for c in range(8):
                        nc.tensor.matmul(
                            ps[:], wk_v[:, c, jsl], kTt_v[:, c, :],
                            start=(c == 0), stop=(c == 7),
                        )
                    nc.vector.tensor_scalar_add(
                        kpT[jt][sb // 2][:, (sb % 2) * 512 : (sb % 2 + 1) * 512],
                        ps[:], biasqk[:, 2 + jt : 3 + jt]
                    )

        # ---- attention + output projection ----
        # Chunk-level software pipeline: per t-chunk the PE stream carries
        # scores(i) for both heads (concurrent row-groups), then attnV(i-1)
        # for both heads, plus occasional "extras" (Wo / q-proj / v-proj /
        # mask prefetch). ACT (exp) is the pacing engine; this keeps it fed
        # every chunk while the PE stays dense enough to hold HAM at 8/8.
        with (
            tc.tile_pool(name="vstream", bufs=3) as vstream,
            tc.tile_pool(name="maskp", bufs=2) as maskp,
            tc.tile_pool(name="ptp", bufs=2) as ptp,
            tc.tile_pool(name="smallp", bufs=2) as smallp,
            tc.tile_pool(name="outp", bufs=3) as outp,
            tc.tile_pool(name="scp", bufs=2, space="PSUM") as scp,
        ):
            mtiles = {}

            def emit_mask_dma(sb):
                sl = slice(sb * 512, (sb + 1) * 512)
                mT = maskp.tile([128, NC_T * 512], bf, tag="mT", name=f"mT{sb}")
                nc.gpsimd.dma_start(
                    mT[:].rearrange("p (c s) -> p c s", c=NC_T),
                    T["maskT"][:, sl].rearrange("(c p) s -> p c s", p=128),
                )
                mtiles[sb] = mT

            vtts = {}

            def emit_vdma(tb):
                sl = slice(tb * 512, (tb + 1) * 512)
                vTt = vstream.tile([128, 8 * 512], bf, tag="vTt", name=f"vTt{tb}")
                nc.gpsimd.dma_start(
                    vTt[:].rearrange("p (c s) -> p c s", c=8),
                    T["vT"][:, sl].rearrange("(c p) s -> p c s", p=128),
                )
                vtts[tb] = vTt[:].rearrange("p (c t) -> p c t", c=8)

            def emit_vproj_tb(tb):
                vTt_v = vtts[tb]
                if tb + 2 < NSB:
                    emit_vdma(tb + 2)
                for tt in range(4):
                    chunk = tb * 4 + tt
                    ps = bigp.tile([128, 512], f32, tag="big", name=f"pv{chunk}")
                    for c in range(8):
                        nc.tensor.matmul(
                            ps[:, 0:JC],
                            vTt_v[:, c, tt * 128 : (tt + 1) * 128],
                            wv_v[:, c, :],
                            start=(c == 0), stop=(c == 7),
                        )
                    vt = vpc[chunk]
                    nc.gpsimd.memset(
                        vt[:].rearrange("p (h d) -> p h d", d=65)[:, :, 64:65],
                        1.0,
                    )
                    dst = vt[:].rearrange("p (h d) -> p h d", h=HPC)[:, :, 0:DK]
                    src = ps[:, 0:JC].rearrange("p (h d) -> p h d", h=HPC)
                    nc.vector.tensor_copy(dst, src)

            def emit_wo_group(sb, st, mt):
                s0 = sb * 512 + st * 128
                msl = slice(mt * 512, (mt + 1) * 512)
                pw = bigp.tile([128, 512], f32, tag="big", name=f"pw{sb}_{st}_{mt}")
                for kc in range(2):
                    nc.tensor.matmul(
                        pw[:],
                        concatT[kc][:, s0 : s0 + 128],
                        wo[kc][:, msl],
                        start=(kc == 0), stop=(kc == 1),
                    )
                ot = outp.tile([128, 512], f32, tag="ot", name=f"ot{sb}_{st}_{mt}")
                nc.vector.tensor_copy(ot[:], pw[:])
                nc.sync.dma_start(T["out_p"][s0 : s0 + 128, msl], ot[:])

            def emit_norm(sb, pair, po2):
                sl = slice(sb * 512, (sb + 1) * 512)
                for h2 in range(2):
                    h = pair * 2 + h2
                    psl = slice(h2 * 64, h2 * 64 + 64)
                    po = po2[h2]
                    rc0 = smallp.tile([1, 512], f32, tag="rc0", name=f"rc0_{sb}_{h}")
                    nc.vector.tensor_copy(rc0[:], po[64:65, :])
                    rc = smallp.tile([1, 512], f32, tag="rc", name=f"rc{sb}_{h}")
                    nc.vector.reciprocal_approx_fast(rc[:], rc0[:])
                    rb = smallp.tile([64, 512], f32, tag="rb", name=f"rb{sb}_{h}")
                    nc.gpsimd.partition_broadcast(rb[:], rc[:], channels=64)
                    nc.vector.tensor_mul(
                        concatT[pair][psl, sl], po[0:64, :], rb[:]
                    )

            emit_mask_dma(0)
            emit_vdma(0)
            emit_vdma(1)
            extras = []
            po2L = None
            prev = None        # (sb, pair, Pt, po2)
            for sb in range(NSB):
                for pair in range(2):
                    last_it = (sb == NSB - 1 and pair == 1)
                    if sb == 0:
                        extras.append(lambda t=2 * pair: emit_vproj_tb(t))
                        extras.append(lambda t=2 * pair + 1: emit_vproj_tb(t))

                    Pt = ptp.tile(
                        [128, 2 * NC_T * 512], bf, tag="Pt", name=f"Pt{sb}_{pair}"
                    )
                    pv = Pt[:].rearrange("p (c h s) -> p c h s", c=NC_T, h=2)
                    mv = mtiles[sb][:].rearrange("p (c s) -> p c s", c=NC_T)
                    if prev is not None:
                        po2 = [
                            bigp.tile([128, 512], f32, tag="big",
                                      name=f"av{prev[0]}_{prev[1]}_{h2}")
                            for h2 in range(2)
                        ]
                    for c in range(NC_T):
                        ps = scp.tile(
                            [128, 1024], f32, tag="sc", name=f"sc{sb}_{pair}_{c}"
                        )
                        for h2 in range(2):
                            psl = slice(h2 * 64, h2 * 64 + 64)
                            nc.tensor.matmul(
                                ps[:, h2 * 512 : (h2 + 1) * 512],
                                kpT[pair][c // 8][psl, (c % 8) * 128 : (c % 8 + 1) * 128],
                                qpS[pair][sb][psl, :],
                                start=True, stop=True,
                            )
                        nc.scalar.activation(
                            Pt[:, c * 1024 : (c + 1) * 1024],
                            ps[:], Exp, scale=SCALE,
                        )
                        if prev is not None:
                            psb, ppair, pPt, _ = prev
                            for h2 in range(2):
                                h = ppair * 2 + h2
                                nc.tensor.matmul(
                                    po2[h2][0:65, :],
                                    vpc[c][:, h * 65 : h * 65 + 65],
                                    pPt[:, (2 * c + h2) * 512 : (2 * c + h2 + 1) * 512],
                                    start=(c == 0), stop=(c == NC_T - 1),
                                )
                        if last_it and c >= 8:
                            if c == 8:
                                po2L = [
                                    bigp.tile([128, 512], f32, tag="big",
                                              name=f"avL_{h2}")
                                    for h2 in range(2)
                                ]
                            cc = c - 8
                            for h2 in range(2):
                                h = pair * 2 + h2
                                nc.tensor.matmul(
                                    po2L[h2][0:65, :],
                                    vpc[cc][:, h * 65 : h * 65 + 65],
                                    Pt[:, (2 * cc + h2) * 512 : (2 * cc + h2 + 1) * 512],
                                    start=(cc == 0), stop=False,
                                )
                        if c == 7 or c == NC_T - 1:
                            half = slice(0, 8) if c == 7 else slice(8, NC_T)
                            for h2 in range(2):
                                nc.vector.tensor_mul(
                                    pv[:, half, h2, :], pv[:, half, h2, :],
                                    mv[:, half, :],
                                )
                        if c == 1 and pair == 0 and sb + 1 < NSB:
                            emit_mask_dma(sb + 1)
                            emit_qdma(sb + 1)
                        elif c in (1, 3) and pair == 1 and sb + 1 < NSB:
                            emit_qproj_jt(sb + 1, c // 2)
                        elif extras and c % 2 == 1 and (c >= 5 or (pair == 0 and c >= 3)):
                            extras.pop(0)()
                    if prev is not None:
                        emit_norm(prev[0], prev[1], po2)
                        if prev[1] == 1:
                            for st in range(4):
                                for mt in range(2):
                                    extras.append(
                                        lambda s=prev[0], a=st, b=mt:
                                        emit_wo_group(s, a, b)
                                    )
                    prev = (sb, pair, Pt, None)
            # tail: finish attnv(3,1) chunks 8..15, then norm + final Wo
            psb, ppair, pPt, _ = prev
            for c in range(8, NC_T):
                for h2 in range(2):
                    h = ppair * 2 + h2
                    nc.tensor.matmul(
                        po2L[h2][0:65, :],
                        vpc[c][:, h * 65 : h * 65 + 65],
                        pPt[:, (2 * c + h2) * 512 : (2 * c + h2 + 1) * 512],
                        start=False, stop=(c == NC_T - 1),
                    )
                if extras and c % 2 == 1:
                    extras.pop(0)()
            emit_norm(psb, ppair, po2L)
            for fn in extras:
                fn()
            for st in range(4):
                for mt in range(2):
                    emit_wo_group(NSB - 1, st, mt)


def build_nc():
    nc = bacc.Bacc("TRN2", target_bir_lowering=False, debug=False)
    names = {}
    def din(name, shape, dt):
        names[name] = nc.dram_tensor(name, shape, dt, kind="ExternalInput").ap()
    din("qT", [D, S], bf)
    din("kT", [D, S], bf)
    din("vT", [D, S], bf)
    din("maskT", [S, S], bf)
    din("wqT", [D, JC], bf)
    din("wkT", [D, JC], bf)
    din("wvT", [D, JC], bf)
    din("woT", [JC, D], bf)
    din("biasqk", [128, 4], f32)
    names["out_p"] = nc.dram_tensor(
        "out_p", [S, D], f32, kind="ExternalOutput"
    ).ap()
    with tile_mod.TileContext(nc) as tc:
        _emit(tc, names)
    nc.compile()
    return nc


_NC = None


def prep_inputs(q, k, v, mask, Wq, bq, Wk, bk, Wv, bv, Wo, bo):
    q = np.asarray(q, F32)
    k = np.asarray(k, F32)
    v = np.asarray(v, F32)
    mask = np.asarray(mask)
    Wq, Wk, Wv, Wo = (np.asarray(w, F32) for w in (Wq, Wk, Wv, Wo))
    bq, bk, bv, bo = (np.asarray(b_, F32) for b_ in (bq, bk, bv, bo))

    maskT = np.ascontiguousarray(mask[0, 0].T).astype(BF16)
    qT = [np.ascontiguousarray(q[b_].T).astype(BF16) for b_ in range(B)]
    kT = [np.ascontiguousarray(k[b_].T).astype(BF16) for b_ in range(B)]
    vT = [np.ascontiguousarray(v[b_].T).astype(BF16) for b_ in range(B)]

    in_maps = []
    for c in range(N_CORES):
        b_, g = c // 4, c % 4
        js = slice(g * JC, (g + 1) * JC)
        biasqk = np.stack(
            [bq[js][:128], bq[js][128:], bk[js][:128], bk[js][128:]], axis=1
        ).astype(F32)
        in_maps.append(
            {
                "qT": qT[b_],
                "kT": kT[b_],
                "vT": vT[b_],
                "maskT": maskT,
                "wqT": np.ascontiguousarray(Wq[js, :].T).astype(BF16),
                "wkT": np.ascontiguousarray(Wk[js, :].T).astype(BF16),
                "wvT": np.ascontiguousarray(Wv[js, :].T).astype(BF16),
                "woT": np.ascontiguousarray(Wo[:, js].T).astype(BF16),
                "biasqk": np.ascontiguousarray(biasqk),
            }
        )
    # bv contributes a constant (softmax rows sum to 1): out += Wo @ bv + bo
    bias_out = (Wo @ bv + bo).astype(F32)
    return in_maps, bias_out


def run_prepped(in_maps, bias_out, trace=False, **kw):
    global _NC
    if _NC is None:
        _NC = build_nc()
    res = run_bass_kernel_spmd(
        _NC, in_maps, list(range(N_CORES)), trace=trace, **kw
    )
    out = np.zeros((B, S, D), F32)
    for c in range(N_CORES):
        out[c // 4] += res.results[c]["out_p"]
    out += bias_out[None, None, :]
    return out, res


def kernel(q, k, v, mask, Wq, bq, Wk, bk, Wv, bv, Wo, bo):
    in_maps, bias_out = prep_inputs(
        q, k, v, mask, Wq, bq, Wk, bk, Wv, bv, Wo, bo
    )
    out, _ = run_prepped(in_maps, bias_out)
    return out
